# revision 3
# baseline (speedup 1.0000x reference)
"""Binarized 3x3 conv (NCHW, VALID, stride 1) on 8 Trainium2 NeuronCores.

Reference: out = conv2d(X, sign(W)) with X [32,256,56,56] f32, W [256,256,3,3]
f32 (OIHW), out [32,256,54,54].

Strategy (data-parallel over batch, weights replicated):
  - Each of the 8 cores gets 4 images. No collectives.
  - Host pre-transposes W to [kh*kw, ci, co] (pure layout, no arithmetic);
    the device binarizes it: sign(w) in {-1,+1} is held as +-0.5 in bf16
    (exact), and the final PSUM->SBUF copy scales by 2 (exact).
  - X is cast f32->bf16 on device; matmuls run in bf16 (weights exact, so the
    only error is the X rounding: ~1e-3 relative on the conv output).
  - The conv is 9 shifted matmuls: out[co, h, w] += WbT[ci, co].T @ X[ci,
    (h+kh)*56 + (w+kw)].  PSUM tile = [128 co, 9 rows x 56 cols] (504 f32 <=
    one 2KB bank).  Rows are computed 56 wide so every rhs is one contiguous
    504-elem slice of the full-image SBUF tensor; the 2 garbage columns per
    row are dropped by the strided PSUM->SBUF copy.
  - Per (image, co_tile): 6 row-groups x 18 accumulating matmuls
    [K=128, M=128, N=504]; weight loop outer, group loop inner so the
    stationary operand is reused across 6 matmuls.
"""

import numpy as np

_N, _C, _H, _W = 32, 256, 56, 56
_CO, _KH, _KW = 256, 3, 3
_HO, _WO = 54, 54
_NCORES = 8
_NPC = _N // _NCORES  # images per core

_R = 9             # output rows per PSUM group
_G = _HO // _R     # 6 row groups
_NF = _R * _W      # 504 = matmul free size
_PAD = 8           # bf16 image pad so the last rhs slice stays in bounds


def build_conv_bass(npc=_NPC):
    import concourse.mybir as mybir
    import concourse.tile as tile
    from concourse import bacc

    fp32 = mybir.dt.float32
    bf16 = mybir.dt.bfloat16

    nc = bacc.Bacc("TRN2", target_bir_lowering=False, debug=False)

    x_in = nc.dram_tensor("x", [npc, _C, _H, _W], fp32, kind="ExternalInput")
    w_in = nc.dram_tensor("w", [_KH * _KW, _C, _CO], fp32, kind="ExternalInput")
    out = nc.dram_tensor("out", [npc, _CO, _HO, _WO], fp32, kind="ExternalOutput")

    n_ci = _C // 128   # 2
    n_co = _CO // 128  # 2

    with tile.TileContext(nc) as tc:
        with (
            tc.tile_pool(name="wstage", bufs=2) as wstage_pool,
            tc.tile_pool(name="wb", bufs=_KH * _KW * n_ci) as wb_pool,
            tc.tile_pool(name="xf", bufs=2) as xf_pool,
            tc.tile_pool(name="xb", bufs=2 * n_ci) as xb_pool,
            tc.tile_pool(name="ob", bufs=6) as ob_pool,
            tc.tile_pool(name="ps", bufs=8, space="PSUM") as ps_pool,
        ):
            # ---- weight prep: load f32 [ci,co] slabs, binarize to +-0.5 bf16
            wb = {}
            for khw in range(_KH * _KW):
                for ci_t in range(n_ci):
                    stage = wstage_pool.tile([128, _CO], fp32)
                    nc.sync.dma_start(
                        stage[:], w_in[khw, ci_t * 128 : (ci_t + 1) * 128, :]
                    )
                    wt = wb_pool.tile([128, _CO], bf16)
                    # (w >= 0) -> {1,0}; minus 0.5 -> {+0.5,-0.5} == sign(w)/2
                    nc.vector.tensor_scalar(
                        wt[:], stage[:], 0.0, 0.5,
                        mybir.AluOpType.is_ge, mybir.AluOpType.subtract,
                    )
                    wb[khw, ci_t] = wt

            # ---- main loop over images
            for n in range(npc):
                xb = {}
                for ci_t in range(n_ci):
                    xf = xf_pool.tile([128, _H * _W], fp32)
                    nc.sync.dma_start(
                        xf[:], x_in[n, ci_t * 128 : (ci_t + 1) * 128, :, :]
                    )
                    xt = xb_pool.tile([128, _H * _W + _PAD], bf16)
                    nc.vector.memset(xt[:, _H * _W :], 0.0)
                    nc.vector.tensor_copy(xt[:, 0 : _H * _W], xf[:])
                    xb[ci_t] = xt

                for co_t in range(n_co):
                    psts = [
                        ps_pool.tile([128, _R, _W], fp32, name="pst", tag="pst")
                        for _ in range(_G)
                    ]
                    for ci_t in range(n_ci):
                        for kh in range(_KH):
                            for kw in range(_KW):
                                w_ap = wb[kh * _KW + kw, ci_t][
                                    :, co_t * 128 : (co_t + 1) * 128
                                ]
                                first = ci_t == 0 and kh == 0 and kw == 0
                                last = (
                                    ci_t == n_ci - 1 and kh == _KH - 1 and kw == _KW - 1
                                )
                                for g in range(_G):
                                    base = (g * _R + kh) * _W + kw
                                    nc.tensor.matmul(
                                        psts[g][:, :, :],
                                        w_ap,
                                        xb[ci_t][:, base : base + _NF],
                                        start=first,
                                        stop=last,
                                    )
                    for g in range(_G):
                        ob = ob_pool.tile([128, _R, _WO], fp32)
                        # x2 undoes the +-0.5 weight encoding (exact)
                        nc.scalar.mul(ob[:], psts[g][:, :, 0:_WO], 2.0)
                        nc.sync.dma_start(
                            out[
                                n,
                                co_t * 128 : (co_t + 1) * 128,
                                g * _R : (g + 1) * _R,
                                :,
                            ],
                            ob[:],
                        )

    nc.compile()
    return nc


_CACHED_NC = None


def _host_weight_layout(W):
    # OIHW [co,ci,kh,kw] -> [kh*kw, ci, co]; layout only, no arithmetic.
    return np.ascontiguousarray(
        np.transpose(np.asarray(W, dtype=np.float32), (2, 3, 1, 0)).reshape(
            _KH * _KW, _C, _CO
        )
    )


def kernel(X, W):
    from concourse.bass_utils import run_bass_kernel_spmd

    global _CACHED_NC
    if _CACHED_NC is None:
        _CACHED_NC = build_conv_bass(_NPC)
    nc = _CACHED_NC

    X = np.asarray(X, dtype=np.float32)
    Wt = _host_weight_layout(W)

    in_maps = [
        {"x": X[c * _NPC : (c + 1) * _NPC], "w": Wt} for c in range(_NCORES)
    ]
    res = run_bass_kernel_spmd(nc, in_maps, core_ids=list(range(_NCORES)))
    return np.concatenate([res.results[c]["out"] for c in range(_NCORES)], axis=0)


# revision 10
# speedup vs baseline: 64271.7067x; 64271.7067x over previous
"""Binarized 3x3 conv (NCHW, VALID, stride 1) on 8 Trainium2 NeuronCores.

Reference: out = conv2d(X, sign(W)) with X [32,256,56,56] f32, W [256,256,3,3]
f32 (OIHW), out [32,256,54,54].

Strategy (data-parallel over batch, weights replicated):
  - Each of the 8 cores gets 4 images. No collectives.
  - Host pre-transposes W to [kh*kw, ci, co] (pure layout, no arithmetic);
    the device binarizes it: sign(w) in {-1,+1} is held as +-0.5 in bf16
    (exact), and the final PSUM->SBUF copy scales by 2 (exact).
  - X is cast f32->bf16 on device; matmuls run in bf16 (weights exact, so the
    only error is the X rounding: ~1e-3 relative on the conv output).
  - The conv is 9 shifted matmuls: out[co, h, w] += WbT[ci, co].T @ X[ci,
    (h+kh)*56 + (w+kw)].  PSUM tile = [128 co, 9 rows x 56 cols] (504 f32 <=
    one 2KB bank).  Rows are computed 56 wide so every rhs is one contiguous
    504-elem slice of the full-image SBUF tensor; the 2 garbage columns per
    row are dropped by the strided PSUM->SBUF copy.
  - Per (image, co_tile): 6 row-groups x 18 accumulating matmuls
    [K=128, M=128, N=504]; weight loop outer, group loop inner so the
    stationary operand is reused across 6 matmuls.
  - Input DMAs ride the SP HWDGE ring, output DMAs the ACT ring.
"""

import numpy as np

_N, _C, _H, _W = 32, 256, 56, 56
_CO, _KH, _KW = 256, 3, 3
_HO, _WO = 54, 54
_NCORES = 8
_NPC = _N // _NCORES  # images per core

_R = 9             # output rows per PSUM group
_G = _HO // _R     # 6 row groups
_NF = _R * _W      # 504 = matmul free size
_PAD = 8           # bf16 image pad so the last rhs slice stays in bounds


def build_conv_bass(npc=_NPC, reps=1, free2d=True, w_on_act=True, cast_chunks=6):
    import concourse.mybir as mybir
    import concourse.tile as tile
    from concourse import bacc

    fp32 = mybir.dt.float32
    bf16 = mybir.dt.bfloat16

    nc = bacc.Bacc("TRN2", target_bir_lowering=False, debug=False)

    x_in = nc.dram_tensor("x", [npc, _C, _H, _W], fp32, kind="ExternalInput")
    w_in = nc.dram_tensor("w", [_KH * _KW, _C, _CO], fp32, kind="ExternalInput")
    out = nc.dram_tensor("out", [npc, _CO, _HO, _WO], fp32, kind="ExternalOutput")

    n_ci = _C // 128   # 2
    n_co = _CO // 128  # 2
    nk = _KH * _KW     # 9

    with tile.TileContext(nc) as tc:
        with (
            tc.tile_pool(name="wstage", bufs=2) as wstage_pool,
            tc.tile_pool(name="wb", bufs=n_ci) as wb_pool,
            tc.tile_pool(name="xf", bufs=2) as xf_pool,
            tc.tile_pool(name="xb", bufs=2 * n_ci) as xb_pool,
            tc.tile_pool(name="ob", bufs=3) as ob_pool,
            tc.tile_pool(name="ps", bufs=8, space="PSUM") as ps_pool,
        ):
            # ---- weight prep: one DMA + one binarize per ci tile.
            # wb[ci_t][:, khw, co] = 0.5*sign(W[co, ci, khw]) in bf16 (exact)
            wb = {}
            w_dma_eng = nc.scalar if w_on_act else nc.sync
            for ci_t in range(n_ci):
                stage = wstage_pool.tile([128, nk, _CO], fp32)
                w_dma_eng.dma_start(
                    stage[:],
                    w_in[:, ci_t * 128 : (ci_t + 1) * 128, :].rearrange(
                        "k c o -> c k o"
                    ),
                )
                wt = wb_pool.tile([128, nk, _CO], bf16)
                # (w >= 0) -> {1,0}; minus 0.5 -> {+0.5,-0.5} == sign(w)/2
                nc.vector.tensor_scalar(
                    wt[:], stage[:], 0.0, 0.5,
                    mybir.AluOpType.is_ge, mybir.AluOpType.subtract,
                )
                wb[ci_t] = wt

            # ---- main loop over images
            for rep in range(reps):
                for n in range(npc):
                    xb = {}
                    for ci_t in range(n_ci):
                        xf = xf_pool.tile([128, _H * _W], fp32)
                        nc.sync.dma_start(
                            xf[:], x_in[n, ci_t * 128 : (ci_t + 1) * 128, :, :]
                        )
                        xt = xb_pool.tile([128, _H * _W + _PAD], bf16)
                        nc.vector.memset(xt[:, _H * _W :], 0.0)
                        # chunked cast so the first matmuls start sooner
                        hw = _H * _W
                        step = -(-hw // cast_chunks)
                        for s in range(0, hw, step):
                            e = min(s + step, hw)
                            nc.vector.tensor_copy(xt[:, s:e], xf[:, s:e])
                        xb[ci_t] = xt

                    for co_t in range(n_co):
                        pshape = [128, _R, _WO] if free2d else [128, _R, _W]
                        psts = [
                            ps_pool.tile(pshape, fp32, name="pst", tag="pst")
                            for _ in range(_G)
                        ]
                        for ci_t in range(n_ci):
                            for kh in range(_KH):
                                for kw in range(_KW):
                                    w_ap = wb[ci_t][
                                        :, kh * _KW + kw,
                                        co_t * 128 : (co_t + 1) * 128,
                                    ]
                                    first = ci_t == 0 and kh == 0 and kw == 0
                                    last = (
                                        ci_t == n_ci - 1
                                        and kh == _KH - 1
                                        and kw == _KW - 1
                                    )
                                    for g in range(_G):
                                        base = (g * _R + kh) * _W + kw
                                        if free2d:
                                            rhs = xb[ci_t][
                                                :, base : base + _NF
                                            ].rearrange(
                                                "p (r w) -> p r w", r=_R
                                            )[:, :, 0:_WO]
                                        else:
                                            rhs = xb[ci_t][:, base : base + _NF]
                                        nc.tensor.matmul(
                                            psts[g][:, :, :],
                                            w_ap,
                                            rhs,
                                            start=first,
                                            stop=last,
                                        )
                        ob = ob_pool.tile([128, _HO, _WO], fp32)
                        for g in range(_G):
                            # x2 undoes the +-0.5 weight encoding (exact)
                            nc.scalar.mul(
                                ob[:, g * _R : (g + 1) * _R, :],
                                psts[g][:, :, 0:_WO] if not free2d else psts[g][:],
                                2.0,
                            )
                        nc.scalar.dma_start(
                            out[n, co_t * 128 : (co_t + 1) * 128, :, :], ob[:]
                        )

    nc.compile()
    return nc


_CACHED_NC = None


def _host_weight_layout(W):
    # OIHW [co,ci,kh,kw] -> [kh*kw, ci, co]; layout only, no arithmetic.
    return np.ascontiguousarray(
        np.transpose(np.asarray(W, dtype=np.float32), (2, 3, 1, 0)).reshape(
            _KH * _KW, _C, _CO
        )
    )


def kernel(X, W):
    import os

    # NTFF tracing is unavailable under this axon image (antenv.axon_hooks
    # missing); make sure a stray BASS_TRACE can't route us into it.
    os.environ["BASS_NEVER_TRACE"] = "1"
    from concourse.bass_utils import run_bass_kernel_spmd

    global _CACHED_NC
    if _CACHED_NC is None:
        _CACHED_NC = build_conv_bass(_NPC)
    nc = _CACHED_NC

    X = np.asarray(X, dtype=np.float32)
    Wt = _host_weight_layout(W)

    in_maps = [
        {"x": X[c * _NPC : (c + 1) * _NPC], "w": Wt} for c in range(_NCORES)
    ]
    res = run_bass_kernel_spmd(nc, in_maps, core_ids=list(range(_NCORES)))
    return np.concatenate([res.results[c]["out"] for c in range(_NCORES)], axis=0)
